# revision 2
# baseline (speedup 1.0000x reference)
"""ConvBnA_int kernel for Trainium2 (Bass/Tile), 8 NeuronCores — fp8 DoubleRow.

Problem: y = clip((conv3x3(x, w, pad=1) + t) >> (-n), act_min, act_max).astype(int8)
  x: (32, 128, 56, 56) f32 (integer values 0..127)
  w: (256, 128, 3, 3) f32 (integer values -128..127)

Strategy:
  - Data-parallel over batch: 4 images per core, 8 cores, no communication.
  - Subtractive-Karatsuba limb split in fp8 e4m3 (exact: all limb values are
    small integers, products accumulate exactly in fp32 PSUM):
      x = 16a + b   (a in 0..8,  b in -8..7)
      w = 16c + d   (c in -8..8, d in -8..7)
      x*w = 17*(16*a*c + b*d) - 16*(a-b)*(c-d)     [verified exhaustively]
    Per output tile two PSUM accumulators:
      Q_a = conv(16a, c) + conv(b, d)    -> 9 DoubleRow MMs (same-tap pairs)
      Q_b = conv(a-b, c-d)               -> 5 DoubleRow MMs
      y   = 17*Q_a - 16*Q_b + t, then >>, clamp.
  - fp8 DoubleRow matmul contracts 256 rows (2 slots x 128 cin) per
    instruction at 0.5 cycles/output-col: 14 MMs/tile vs 18 bf16-equivalent.
    Q_a pairs slot0=16a-array with weight c, slot1=b-array with weight d at
    the same tap (slot stride = 4096 = array pitch). Q_b pairs taps
    (k,0)+(k,1) via a pre-shifted copy of (a-b) (slot stride 4096), taps
    (0,2)+(1,2) via slot stride 64 (row pitch), and tap (2,2) with a zeroed
    slot1.
  - Images live in SBUF as [128, 2, 64x64] fp8 canvases (zero border, row
    pitch 64 so DMA runs are 4KB and slot strides stay 16-aligned).
  - Post per tile [128, 8x56]: ACT u=i32(Q_a*16+t), ACT v=i32(Q_b*-15),
    GPSIMD z=u+v, DVE shift, DVE clamp->int8 (engines balanced so the PE
    stream of DoubleRow MMs is the critical path).
"""

import numpy as np
import ml_dtypes

B, CIN, COUT, H, W, K = 32, 128, 256, 56, 56, 3
N_CORES = 8
B_LOC = B // N_CORES          # 4 images per core
PW = 64                       # padded row pitch (56 + 2 border + 6 pad)
PADN = 64 * PW                # 4096 elems per slot per partition
ROWS_PER_TILE = 8
NTILE = H // ROWS_PER_TILE    # 7 spatial tiles
TILE_N = ROWS_PER_TILE * W    # 448
NQ = H * W                    # 3136
CTILES = COUT // 128          # 2

F8 = ml_dtypes.float8_e4m3
WARM_MMS = 18       # warm-up matmuls ahead of the real stream
WARM_N = 448        # output cols per warm-up matmul
CONST_ENG = "scalar"  # which queue carries tv/sv/amin/amax
TAIL_DVE = True     # last tile's clamp on DVE instead of Pool

_CACHE = {}


def _build_nc():
    import concourse.mybir as mybir
    import concourse.tile as tile
    from concourse import bacc
    from concourse.ap import AP

    dt = mybir.dt
    DR = mybir.MatmulPerfMode.DoubleRow
    nc = bacc.Bacc(
        "TRN2", target_bir_lowering=False, debug=False, num_devices=N_CORES
    )

    xa = nc.dram_tensor("xa", [B_LOC, CIN, 2 * PADN], dt.float8e4, kind="ExternalInput")
    xb = nc.dram_tensor("xb", [B_LOC, CIN, 2 * PADN], dt.float8e4, kind="ExternalInput")
    wqa = nc.dram_tensor("wqa", [CIN, 9 * 2 * CTILES * 128], dt.float8e4, kind="ExternalInput")
    wqb = nc.dram_tensor("wqb", [CIN, 5 * 2 * CTILES * 128], dt.float8e4, kind="ExternalInput")
    tv = nc.dram_tensor("tv", [128, CTILES], dt.float32, kind="ExternalInput")
    sv = nc.dram_tensor("sv", [128, CTILES], dt.int32, kind="ExternalInput")
    amin = nc.dram_tensor("amin", [128, CTILES], dt.float32, kind="ExternalInput")
    amax = nc.dram_tensor("amax", [128, CTILES], dt.float32, kind="ExternalInput")
    out = nc.dram_tensor("out", [B_LOC, COUT, NQ], dt.int8, kind="ExternalOutput")

    with tile.TileContext(nc) as tc:
        with (
            tc.tile_pool(name="const", bufs=1) as cpool,
            tc.tile_pool(name="xin", bufs=2) as xin_pool,
            tc.tile_pool(name="psum", bufs=4, space="PSUM") as pspool,
            tc.tile_pool(name="uv", bufs=8) as uv_pool,
            tc.tile_pool(name="zz", bufs=4) as zz_pool,
            tc.tile_pool(name="o8", bufs=6) as o8_pool,
        ):
            # weights + per-channel vectors on the scalar HWDGE queue
            # (parallel with x loads on sync); wqa split so the first Qa taps
            # and wqb land early (transfers serialize on the DMA device)
            # per-couttile weight tiles, contiguous on both sides; c=0's
            # weights land first so the first units start sooner
            wqa_sb = cpool.tile([CIN, 9, 2, CTILES, 128], dt.float8e4)
            nc.scalar.dma_start(
                wqa_sb[:],
                wqa[:, :].rearrange("p (t s c m) -> p t s c m", t=9, s=2, c=CTILES),
            )
            wqb_sb = cpool.tile([CIN, 5, 2, CTILES, 128], dt.float8e4)
            nc.scalar.dma_start(
                wqb_sb[:],
                wqb[:, :].rearrange("p (t s c m) -> p t s c m", t=5, s=2, c=CTILES),
            )
            ceng = getattr(nc, CONST_ENG)
            tv_sb = cpool.tile([128, CTILES], dt.float32)
            ceng.dma_start(tv_sb[:], tv[:, :])
            sv_sb = cpool.tile([128, CTILES], dt.int32)
            ceng.dma_start(sv_sb[:], sv[:, :])
            amin_sb = cpool.tile([128, CTILES], dt.float32)
            ceng.dma_start(amin_sb[:], amin[:, :])
            amax_sb = cpool.tile([128, CTILES], dt.float32)
            ceng.dma_start(amax_sb[:], amax[:, :])

            # PE clock warm-up: a trickle of tiny dependency-free MMs keeps
            # the PE continuously busy from t~0 so the p-state ramp (3us)
            # completes before the real MM stream starts
            junk = cpool.tile([CIN, 2, ROWS_PER_TILE * W], dt.float8e4)
            nc.vector.memset(junk[:].bitcast(mybir.dt.int32), 0.0)

            for b in range(B_LOC):
                xa_sb = xin_pool.tile([CIN, 2, PADN], dt.float8e4)
                xb_sb = xin_pool.tile([CIN, 2, PADN], dt.float8e4)
                # chunk the first image so the first tiles' MMs start early;
                # chunk-major order: the four slot-chunk-0s go first because
                # transfers serialize on the shared DMA device
                bounds = [0, 1024, 2560, PADN] if b == 0 else [0, PADN]
                for lo, hi in zip(bounds[:-1], bounds[1:]):
                    for t_sb, t_dr in ((xa_sb, xa), (xb_sb, xb)):
                        for s in range(2):
                            nc.sync.dma_start(
                                t_sb[:, s, lo:hi],
                                t_dr[b, :, s * PADN + lo : s * PADN + hi],
                            )

                xav = xa_sb.rearrange("p s (h w) -> p s h w", w=PW)
                xbflat = xb_sb[:]
                pstride = xbflat.ap[0][0]

                def mkap(off, sstride):
                    return AP(
                        xbflat.tensor, off,
                        [[pstride, CIN], [sstride, 2],
                         [PW, ROWS_PER_TILE], [1, W]],
                    )

                for c in range(CTILES):
                    for st in range(NTILE):
                        h0 = st * ROWS_PER_TILE
                        psA = pspool.tile([128, ROWS_PER_TILE, W], dt.float32)
                        if b == 0 and c == 0 and st == 0:
                            wn = min(WARM_N, ROWS_PER_TILE * W)
                            for _ in range(WARM_MMS):
                                nc.tensor.matmul(
                                    psA[:].rearrange("p h w -> p (h w)")[:, 0:wn],
                                    junk[:, :, 0:128],
                                    junk[:, :, 0:wn],
                                    start=True, stop=True,
                                    perf_mode=DR, skip_group_check=True,
                                )
                        for t9 in range(9):
                            kh, kw = divmod(t9, K)
                            nc.tensor.matmul(
                                psA[:],
                                wqa_sb[:, t9, :, c, :],
                                xav[:, :, h0 + kh : h0 + kh + ROWS_PER_TILE,
                                    kw : kw + W],
                                start=(t9 == 0), stop=(t9 == 8),
                                perf_mode=DR,
                            )
                        psB = pspool.tile([128, ROWS_PER_TILE, W], dt.float32)
                        qb_movs = [
                            (h0 * PW, PADN),            # taps (0,0)+(0,1)
                            ((h0 + 1) * PW, PADN),      # taps (1,0)+(1,1)
                            ((h0 + 2) * PW, PADN),      # taps (2,0)+(2,1)
                            (h0 * PW + 2, PW),          # taps (0,2)+(1,2)
                            ((h0 + 2) * PW + 2, PW),    # tap (2,2) + zero slot
                        ]
                        for t5, (off, sstride) in enumerate(qb_movs):
                            nc.tensor.matmul(
                                psB[:],
                                wqb_sb[:, t5, :, c, :],
                                mkap(off, sstride),
                                start=(t5 == 0), stop=(t5 == 4),
                                perf_mode=DR,
                            )

                        # u = f32(17*Q_a + t)       [ACT]
                        # z = i32((Q_b*-16) + u)    [DVE STT, reads PSUM]
                        # sh = z >> sv              [DVE]
                        # o8 = clamp(sh) -> int8    [GPSIMD]
                        u32 = uv_pool.tile([128, ROWS_PER_TILE, W], dt.float32)
                        nc.scalar.activation(
                            u32[:], psA[:],
                            mybir.ActivationFunctionType.Identity,
                            bias=tv_sb[:, c : c + 1], scale=17.0,
                        )
                        z32 = zz_pool.tile([128, ROWS_PER_TILE, W], dt.int32)
                        nc.vector.scalar_tensor_tensor(
                            z32[:], psB[:], -16.0, u32[:],
                            mybir.AluOpType.mult, mybir.AluOpType.add,
                        )
                        sh32 = zz_pool.tile([128, ROWS_PER_TILE, W], dt.int32)
                        nc.vector.tensor_scalar(
                            sh32[:], z32[:],
                            sv_sb[:, c : c + 1], None,
                            mybir.AluOpType.arith_shift_right,
                        )
                        # batch stores in pairs of spatial tiles
                        if st % 2 == 0:
                            o8 = o8_pool.tile(
                                [128, 2 * ROWS_PER_TILE, W], dt.int8, name="o8"
                            )
                        half = st % 2
                        # last unit's clamp on DVE: shorter tail latency (no
                        # Pool hop after the final DVE shift)
                        clamp_eng = (
                            nc.vector
                            if (TAIL_DVE and b == B_LOC - 1 and c == CTILES - 1
                                and st == NTILE - 1)
                            else nc.gpsimd
                        )
                        clamp_eng.tensor_scalar(
                            o8[:, half * ROWS_PER_TILE : (half + 1) * ROWS_PER_TILE],
                            sh32[:],
                            amax_sb[:, c : c + 1], amin_sb[:, c : c + 1],
                            mybir.AluOpType.min, mybir.AluOpType.max,
                        )
                        if st % 2 == 1 or st == NTILE - 1:
                            npair = 1 if st == NTILE - 1 and st % 2 == 0 else 2
                            lo = (st - npair + 1) * TILE_N
                            nc.sync.dma_start(
                                out[b, c * 128 : (c + 1) * 128,
                                    lo : lo + npair * TILE_N]
                                .rearrange("p (h w) -> p h w", w=W),
                                o8[:, : npair * ROWS_PER_TILE],
                            )
    nc.compile()
    return nc


def _f8_lut():
    # uint8 bit patterns for exact small-int -> fp8 e4m3 conversion
    vals = np.arange(-16, 241, dtype=np.int32)
    lut = np.zeros(257, dtype=np.uint8)
    lut[:] = vals.astype(np.float32).astype(F8).view(np.uint8)
    return lut


def _prep_inputs(x, weight, t, n, act_min, act_max):
    lut = _f8_lut()

    def to_f8(ints):  # int array (>= -16) -> fp8 bytes
        return lut[ints + 16].view(F8)

    xi = x.astype(np.int32)
    a = (xi + 8) >> 4                 # 0..8
    bb = xi - (a << 4)                # -8..7

    def canvas(vals):  # [B, CIN, 56, 56] int32 -> [B, CIN, PADN] int32
        cv = np.zeros((B, CIN, 64, PW), dtype=np.int32)
        cv[:, :, 1 : H + 1, 1 : W + 1] = vals
        return cv.reshape(B, CIN, PADN)

    A2 = canvas(a << 4)
    Bc = canvas(bb)
    D2 = canvas(a - bb)               # -7..16
    D2s = np.zeros_like(D2)
    D2s[:, :, : PADN - 1] = D2[:, :, 1:]
    xa = to_f8(np.concatenate([A2, Bc], axis=2))
    xb = to_f8(np.concatenate([D2, D2s], axis=2))

    wi = weight.astype(np.int32)      # [COUT, CIN, 3, 3]
    c = (wi + 8) >> 4                 # -8..8
    d = wi - (c << 4)                 # -8..7
    e = c - d                         # -15..16

    # wqa[p, tap, slot, ct, m]: slot0 = c (vs 16a), slot1 = d (vs b)
    wqa = np.zeros((CIN, 9, 2, CTILES, 128), dtype=np.int32)
    wqb = np.zeros((CIN, 5, 2, CTILES, 128), dtype=np.int32)
    cT = c.reshape(CTILES, 128, CIN, K, K).transpose(2, 3, 4, 0, 1)   # [p,kh,kw,ct,m]
    dT = d.reshape(CTILES, 128, CIN, K, K).transpose(2, 3, 4, 0, 1)
    eT = e.reshape(CTILES, 128, CIN, K, K).transpose(2, 3, 4, 0, 1)
    for t9 in range(9):
        kh, kw = divmod(t9, K)
        wqa[:, t9, 0] = cT[:, kh, kw]
        wqa[:, t9, 1] = dT[:, kh, kw]
    for k in range(3):
        wqb[:, k, 0] = eT[:, k, 0]
        wqb[:, k, 1] = eT[:, k, 1]
    wqb[:, 3, 0] = eT[:, 0, 2]
    wqb[:, 3, 1] = eT[:, 1, 2]
    wqb[:, 4, 0] = eT[:, 2, 2]
    # wqb[:, 4, 1] stays zero
    wqa_f8 = to_f8(wqa.reshape(CIN, -1))
    wqb_f8 = to_f8(wqb.reshape(CIN, -1))

    def percore_vec(v, dtype):
        return np.ascontiguousarray(v.reshape(CTILES, 128).T).astype(dtype)

    tvv = percore_vec(t, np.float32)
    svv = percore_vec(-n, np.int32)
    amin_v = percore_vec(act_min, np.float32)
    amax_v = percore_vec(act_max, np.float32)
    return xa, xb, wqa_f8, wqb_f8, tvv, svv, amin_v, amax_v


def kernel(x, weight, t, n, act_min, act_max):
    from concourse.bass_utils import run_bass_kernel_spmd

    xa, xb, wqa, wqb, tvv, svv, amin_v, amax_v = _prep_inputs(
        x, weight, t, n, act_min, act_max
    )

    if "nc" not in _CACHE:
        _CACHE["nc"] = _build_nc()
    nc = _CACHE["nc"]

    in_maps = []
    for c in range(N_CORES):
        in_maps.append(
            dict(
                xa=xa[c * B_LOC : (c + 1) * B_LOC],
                xb=xb[c * B_LOC : (c + 1) * B_LOC],
                wqa=wqa, wqb=wqb, tv=tvv, sv=svv, amin=amin_v, amax=amax_v,
            )
        )
    res = run_bass_kernel_spmd(nc, in_maps, core_ids=list(range(N_CORES)))
    outs = [r["out"] for r in res.results]
    full = np.concatenate(outs, axis=0)              # [32, 256, 3136]
    return np.ascontiguousarray(full.reshape(B, COUT, H, W))


# revision 4
# speedup vs baseline: 1.0110x; 1.0110x over previous
"""ConvBnA_int kernel for Trainium2 (Bass/Tile), 8 NeuronCores — fp8 DoubleRow.

Problem: y = clip((conv3x3(x, w, pad=1) + t) >> (-n), act_min, act_max).astype(int8)
  x: (32, 128, 56, 56) f32 (integer values 0..127)
  w: (256, 128, 3, 3) f32 (integer values -128..127)

Strategy:
  - Data-parallel over batch: 4 images per core, 8 cores, no communication.
  - Subtractive-Karatsuba limb split in fp8 e4m3 (exact: all limb values are
    small integers, products accumulate exactly in fp32 PSUM):
      x = 16a + b   (a in 0..8,  b in -8..7)
      w = 16c + d   (c in -8..8, d in -8..7)
      x*w = 17*(16*a*c + b*d) - 16*(a-b)*(c-d)     [verified exhaustively]
    Per output tile two PSUM accumulators:
      Q_a = conv(16a, c) + conv(b, d)    -> 9 DoubleRow MMs (same-tap pairs)
      Q_b = conv(a-b, c-d)               -> 5 DoubleRow MMs
      y   = 17*Q_a - 16*Q_b + t, then >>, clamp.
  - fp8 DoubleRow matmul contracts 256 rows (2 slots x 128 cin) per
    instruction at 0.5 cycles/output-col: 14 MMs/tile vs 18 bf16-equivalent.
    Q_a pairs slot0=16a-array with weight c, slot1=b-array with weight d at
    the same tap (slot stride = 4096 = array pitch). Q_b pairs taps
    (k,0)+(k,1) via a pre-shifted copy of (a-b) (slot stride 4096), taps
    (0,2)+(1,2) via slot stride 64 (row pitch), and tap (2,2) with a zeroed
    slot1.
  - Images live in SBUF as [128, 2, 64x64] fp8 canvases (zero border, row
    pitch 64 so DMA runs are >=512B and slot strides stay 16-aligned; the
    DoubleRow moving-operand slot stride must be a multiple of 16 — probed:
    stride 1 gives wrong results on hardware).
  - Post per tile [128, 8x56]: ACT u=f32(17*Q_a+t), DVE scalar_tensor_tensor
    z=i32((Q_b*-16)+u), DVE shift, GPSIMD clamp->int8 (engines balanced so
    the PE stream of DoubleRow MMs is the critical path; the last unit's
    clamp runs on DVE to shorten the drain tail).
  - Exactness: fp8 operands/products are exact; fp32 rounding can only occur
    for |pre-shift| > 2^24, which after >>5..10 lands far beyond the clamp
    bounds, so the int8 output is still exact (verified 0 mismatches).
"""

import numpy as np
import ml_dtypes

B, CIN, COUT, H, W, K = 32, 128, 256, 56, 56, 3
N_CORES = 8
B_LOC = B // N_CORES          # 4 images per core
PW = 64                       # padded row pitch (56 + 2 border + 6 pad)
PADN = 64 * PW                # 4096 elems per slot per partition
ROWS_PER_TILE = 8
NTILE = H // ROWS_PER_TILE    # 7 spatial tiles
TILE_N = ROWS_PER_TILE * W    # 448
NQ = H * W                    # 3136
CTILES = COUT // 128          # 2

F8 = ml_dtypes.float8_e4m3
WARM_MMS = 18       # warm-up matmuls ahead of the real stream
WARM_N = 448        # output cols per warm-up matmul
CONST_ENG = "scalar"  # which queue carries tv/sv/amin/amax
TAIL_DVE_K = 1      # clamp of the last K units runs on DVE instead of Pool
TAIL_SPLIT = False  # split the final unit's post into two half-tiles
WSPLIT = 9          # wqa DMA split point (9 = single DMA)
CHUNK0 = 1024       # first-image chunk-0 size

_CACHE = {}


def _build_nc():
    import concourse.mybir as mybir
    import concourse.tile as tile
    from concourse import bacc
    from concourse.ap import AP

    dt = mybir.dt
    DR = mybir.MatmulPerfMode.DoubleRow
    nc = bacc.Bacc(
        "TRN2", target_bir_lowering=False, debug=False, num_devices=N_CORES
    )

    xa = nc.dram_tensor("xa", [B_LOC, CIN, 2 * PADN], dt.float8e4, kind="ExternalInput")
    xb = nc.dram_tensor("xb", [B_LOC, CIN, 2 * PADN], dt.float8e4, kind="ExternalInput")
    wqa = nc.dram_tensor("wqa", [CIN, 9 * 2 * CTILES * 128], dt.float8e4, kind="ExternalInput")
    wqb = nc.dram_tensor("wqb", [CIN, 5 * 2 * CTILES * 128], dt.float8e4, kind="ExternalInput")
    tv = nc.dram_tensor("tv", [128, CTILES], dt.float32, kind="ExternalInput")
    sv = nc.dram_tensor("sv", [128, CTILES], dt.int32, kind="ExternalInput")
    amin = nc.dram_tensor("amin", [128, CTILES], dt.float32, kind="ExternalInput")
    amax = nc.dram_tensor("amax", [128, CTILES], dt.float32, kind="ExternalInput")
    out = nc.dram_tensor("out", [B_LOC, COUT, NQ], dt.int8, kind="ExternalOutput")

    with tile.TileContext(nc) as tc:
        with (
            tc.tile_pool(name="const", bufs=1) as cpool,
            tc.tile_pool(name="xin", bufs=2) as xin_pool,
            tc.tile_pool(name="psum", bufs=4, space="PSUM") as pspool,
            tc.tile_pool(name="uv", bufs=8) as uv_pool,
            tc.tile_pool(name="zz", bufs=4) as zz_pool,
            tc.tile_pool(name="o8", bufs=6) as o8_pool,
        ):
            # weights + per-channel vectors on the scalar HWDGE queue
            # (parallel with x loads on sync); wqa split so the first Qa taps
            # and wqb land early (transfers serialize on the DMA device)
            # per-couttile weight tiles, contiguous on both sides; c=0's
            # weights land first so the first units start sooner
            wqa_sb = cpool.tile([CIN, 9, 2, CTILES, 128], dt.float8e4)
            wqa_v = wqa[:, :].rearrange("p (t s c m) -> p t s c m", t=9, s=2, c=CTILES)
            nc.scalar.dma_start(wqa_sb[:, 0:WSPLIT], wqa_v[:, 0:WSPLIT])
            wqb_sb = cpool.tile([CIN, 5, 2, CTILES, 128], dt.float8e4)
            nc.scalar.dma_start(
                wqb_sb[:],
                wqb[:, :].rearrange("p (t s c m) -> p t s c m", t=5, s=2, c=CTILES),
            )
            if WSPLIT < 9:
                nc.scalar.dma_start(wqa_sb[:, WSPLIT:9], wqa_v[:, WSPLIT:9])
            ceng = getattr(nc, CONST_ENG)
            tv_sb = cpool.tile([128, CTILES], dt.float32)
            ceng.dma_start(tv_sb[:], tv[:, :])
            sv_sb = cpool.tile([128, CTILES], dt.int32)
            ceng.dma_start(sv_sb[:], sv[:, :])
            amin_sb = cpool.tile([128, CTILES], dt.float32)
            ceng.dma_start(amin_sb[:], amin[:, :])
            amax_sb = cpool.tile([128, CTILES], dt.float32)
            ceng.dma_start(amax_sb[:], amax[:, :])

            # PE clock warm-up: a trickle of tiny dependency-free MMs keeps
            # the PE continuously busy from t~0 so the p-state ramp (3us)
            # completes before the real MM stream starts
            junk = cpool.tile([CIN, 2, ROWS_PER_TILE * W], dt.float8e4)
            nc.gpsimd.memset(junk[:].bitcast(mybir.dt.int32), 0.0)

            for b in range(B_LOC):
                xa_sb = xin_pool.tile([CIN, 2, PADN], dt.float8e4)
                xb_sb = xin_pool.tile([CIN, 2, PADN], dt.float8e4)
                # chunk the first image so the first tiles' MMs start early;
                # chunk-major order: the four slot-chunk-0s go first because
                # transfers serialize on the shared DMA device. Rows past the
                # padded content (never read / zero-weight-only) are skipped:
                # xa taps read rows <=57 (3712), xb's zero-slot reads <=58.
                XA_END, XB_END = 58 * PW, 59 * PW
                nchunk = 3 if b == 0 else 1
                for ci in range(nchunk):
                    for t_sb, t_dr, end in ((xa_sb, xa, XA_END), (xb_sb, xb, XB_END)):
                        bnds = [0, CHUNK0, 2560, end] if b == 0 else [0, end]
                        lo, hi = bnds[ci], bnds[ci + 1]
                        for s in range(2):
                            nc.sync.dma_start(
                                t_sb[:, s, lo:hi],
                                t_dr[b, :, s * PADN + lo : s * PADN + hi],
                            )

                xav = xa_sb.rearrange("p s (h w) -> p s h w", w=PW)
                xbflat = xb_sb[:]
                pstride = xbflat.ap[0][0]

                def mkap(off, sstride):
                    return AP(
                        xbflat.tensor, off,
                        [[pstride, CIN], [sstride, 2],
                         [PW, ROWS_PER_TILE], [1, W]],
                    )

                for c in range(CTILES):
                    for st in range(NTILE):
                        h0 = st * ROWS_PER_TILE
                        psA = pspool.tile([128, ROWS_PER_TILE, W], dt.float32)
                        if b == 0 and c == 0 and st == 0:
                            wn = min(WARM_N, ROWS_PER_TILE * W)
                            for _ in range(WARM_MMS):
                                nc.tensor.matmul(
                                    psA[:].rearrange("p h w -> p (h w)")[:, 0:wn],
                                    junk[:, :, 0:128],
                                    junk[:, :, 0:wn],
                                    start=True, stop=True,
                                    perf_mode=DR, skip_group_check=True,
                                )
                        for t9 in range(9):
                            kh, kw = divmod(t9, K)
                            nc.tensor.matmul(
                                psA[:],
                                wqa_sb[:, t9, :, c, :],
                                xav[:, :, h0 + kh : h0 + kh + ROWS_PER_TILE,
                                    kw : kw + W],
                                start=(t9 == 0), stop=(t9 == 8),
                                perf_mode=DR,
                            )
                        psB = pspool.tile([128, ROWS_PER_TILE, W], dt.float32)
                        qb_movs = [
                            (h0 * PW, PADN),            # taps (0,0)+(0,1)
                            ((h0 + 1) * PW, PADN),      # taps (1,0)+(1,1)
                            ((h0 + 2) * PW, PADN),      # taps (2,0)+(2,1)
                            (h0 * PW + 2, PW),          # taps (0,2)+(1,2)
                            ((h0 + 2) * PW + 2, PW),    # tap (2,2) + zero slot
                        ]
                        for t5, (off, sstride) in enumerate(qb_movs):
                            nc.tensor.matmul(
                                psB[:],
                                wqb_sb[:, t5, :, c, :],
                                mkap(off, sstride),
                                start=(t5 == 0), stop=(t5 == 4),
                                perf_mode=DR,
                            )

                        # u = f32(17*Q_a + t)       [ACT]
                        # z = i32((Q_b*-16) + u)    [DVE STT, reads PSUM]
                        # sh = z >> sv              [DVE]
                        # o8 = clamp(sh) -> int8    [GPSIMD; DVE for tail]
                        ui = (b * CTILES + c) * NTILE + st
                        units_left = B_LOC * CTILES * NTILE - 1 - ui
                        is_last = units_left == 0
                        # batch stores in pairs of spatial tiles
                        if st % 2 == 0:
                            o8 = o8_pool.tile(
                                [128, 2 * ROWS_PER_TILE, W], dt.int8, name="o8"
                            )
                        half = st % 2
                        rr = (
                            [(0, 4), (4, 8)]
                            if (TAIL_SPLIT and is_last)
                            else [(0, ROWS_PER_TILE)]
                        )
                        for r0, r1 in rr:
                            nr = r1 - r0
                            u32 = uv_pool.tile([128, nr, W], dt.float32, name="u32")
                            nc.scalar.activation(
                                u32[:], psA[:, r0:r1],
                                mybir.ActivationFunctionType.Identity,
                                bias=tv_sb[:, c : c + 1], scale=17.0,
                            )
                            z32 = zz_pool.tile([128, nr, W], dt.int32, name="z32")
                            nc.vector.scalar_tensor_tensor(
                                z32[:], psB[:, r0:r1], -16.0, u32[:],
                                mybir.AluOpType.mult, mybir.AluOpType.add,
                            )
                            sh32 = zz_pool.tile([128, nr, W], dt.int32, name="sh32")
                            nc.vector.tensor_scalar(
                                sh32[:], z32[:],
                                sv_sb[:, c : c + 1], None,
                                mybir.AluOpType.arith_shift_right,
                            )
                            clamp_eng = (
                                nc.vector if units_left < TAIL_DVE_K else nc.gpsimd
                            )
                            clamp_eng.tensor_scalar(
                                o8[:, half * ROWS_PER_TILE + r0
                                   : half * ROWS_PER_TILE + r1],
                                sh32[:],
                                amax_sb[:, c : c + 1], amin_sb[:, c : c + 1],
                                mybir.AluOpType.min, mybir.AluOpType.max,
                            )
                            if TAIL_SPLIT and is_last:
                                lo = st * TILE_N + r0 * W
                                nc.sync.dma_start(
                                    out[b, c * 128 : (c + 1) * 128,
                                        lo : lo + nr * W]
                                    .rearrange("p (h w) -> p h w", w=W),
                                    o8[:, half * ROWS_PER_TILE + r0
                                       : half * ROWS_PER_TILE + r1],
                                )
                        if (st % 2 == 1 or st == NTILE - 1) and not (
                            TAIL_SPLIT and is_last
                        ):
                            npair = 1 if st == NTILE - 1 and st % 2 == 0 else 2
                            lo = (st - npair + 1) * TILE_N
                            nc.sync.dma_start(
                                out[b, c * 128 : (c + 1) * 128,
                                    lo : lo + npair * TILE_N]
                                .rearrange("p (h w) -> p h w", w=W),
                                o8[:, : npair * ROWS_PER_TILE],
                            )
    nc.compile()
    return nc


def _f8_lut():
    # uint8 bit patterns for exact small-int -> fp8 e4m3 conversion
    vals = np.arange(-16, 241, dtype=np.int32)
    lut = np.zeros(257, dtype=np.uint8)
    lut[:] = vals.astype(np.float32).astype(F8).view(np.uint8)
    return lut


def _prep_inputs(x, weight, t, n, act_min, act_max):
    lut = _f8_lut()

    def to_f8(ints):  # int array (>= -16) -> fp8 bytes
        return lut[ints + 16].view(F8)

    xi = x.astype(np.int32)
    a = (xi + 8) >> 4                 # 0..8
    bb = xi - (a << 4)                # -8..7

    def canvas(vals):  # [B, CIN, 56, 56] int32 -> [B, CIN, PADN] int32
        cv = np.zeros((B, CIN, 64, PW), dtype=np.int32)
        cv[:, :, 1 : H + 1, 1 : W + 1] = vals
        return cv.reshape(B, CIN, PADN)

    A2 = canvas(a << 4)
    Bc = canvas(bb)
    D2 = canvas(a - bb)               # -7..16
    D2s = np.zeros_like(D2)
    D2s[:, :, : PADN - 1] = D2[:, :, 1:]
    xa = to_f8(np.concatenate([A2, Bc], axis=2))
    xb = to_f8(np.concatenate([D2, D2s], axis=2))

    wi = weight.astype(np.int32)      # [COUT, CIN, 3, 3]
    c = (wi + 8) >> 4                 # -8..8
    d = wi - (c << 4)                 # -8..7
    e = c - d                         # -15..16

    # wqa[p, tap, slot, ct, m]: slot0 = c (vs 16a), slot1 = d (vs b)
    wqa = np.zeros((CIN, 9, 2, CTILES, 128), dtype=np.int32)
    wqb = np.zeros((CIN, 5, 2, CTILES, 128), dtype=np.int32)
    cT = c.reshape(CTILES, 128, CIN, K, K).transpose(2, 3, 4, 0, 1)   # [p,kh,kw,ct,m]
    dT = d.reshape(CTILES, 128, CIN, K, K).transpose(2, 3, 4, 0, 1)
    eT = e.reshape(CTILES, 128, CIN, K, K).transpose(2, 3, 4, 0, 1)
    for t9 in range(9):
        kh, kw = divmod(t9, K)
        wqa[:, t9, 0] = cT[:, kh, kw]
        wqa[:, t9, 1] = dT[:, kh, kw]
    for k in range(3):
        wqb[:, k, 0] = eT[:, k, 0]
        wqb[:, k, 1] = eT[:, k, 1]
    wqb[:, 3, 0] = eT[:, 0, 2]
    wqb[:, 3, 1] = eT[:, 1, 2]
    wqb[:, 4, 0] = eT[:, 2, 2]
    # wqb[:, 4, 1] stays zero
    wqa_f8 = to_f8(wqa.reshape(CIN, -1))
    wqb_f8 = to_f8(wqb.reshape(CIN, -1))

    def percore_vec(v, dtype):
        return np.ascontiguousarray(v.reshape(CTILES, 128).T).astype(dtype)

    tvv = percore_vec(t, np.float32)
    svv = percore_vec(-n, np.int32)
    amin_v = percore_vec(act_min, np.float32)
    amax_v = percore_vec(act_max, np.float32)
    return xa, xb, wqa_f8, wqb_f8, tvv, svv, amin_v, amax_v


def kernel(x, weight, t, n, act_min, act_max):
    from concourse.bass_utils import run_bass_kernel_spmd

    xa, xb, wqa, wqb, tvv, svv, amin_v, amax_v = _prep_inputs(
        x, weight, t, n, act_min, act_max
    )

    if "nc" not in _CACHE:
        _CACHE["nc"] = _build_nc()
    nc = _CACHE["nc"]

    in_maps = []
    for c in range(N_CORES):
        in_maps.append(
            dict(
                xa=xa[c * B_LOC : (c + 1) * B_LOC],
                xb=xb[c * B_LOC : (c + 1) * B_LOC],
                wqa=wqa, wqb=wqb, tv=tvv, sv=svv, amin=amin_v, amax=amax_v,
            )
        )
    res = run_bass_kernel_spmd(nc, in_maps, core_ids=list(range(N_CORES)))
    outs = [r["out"] for r in res.results]
    full = np.concatenate(outs, axis=0)              # [32, 256, 3136]
    return np.ascontiguousarray(full.reshape(B, COUT, H, W))
